# revision 1
# baseline (speedup 1.0000x reference)
"""Block-causal attention block (RMSnorm + QKV + frame-causal attention + proj)
on 8 TRN2 NeuronCores.

Sharding: sequence-parallel over the 8 frames — core i owns the 1024 queries of
frame i and processes KV blocks for frames 0..i (uniform SPMD program: all 16
half-blocks are processed on every core; future frames are killed by a
per-core additive bias of -1e30 before the exp, so they contribute exp() = 0
to both the numerator and denominator of the softmax).

Layouts are channel-first throughout ([C, seq] with C on partitions), which
makes every contraction a natural PE matmul with no transposes:
  k^T [C, kv]   = Wk' @ xn           (lhsT = wkT chunk, rhs = xn)
  v   [kv, C]   = xn^T @ Wv'^T       (lhsT = xn chunk,  rhs = wvT)
  S^T [kv, q]   = K @ Q^T            (lhsT = k^T chunk, rhs = q^T)
  O^T [C, q]    = V^T @ P^T          (lhsT = v chunk,   rhs = p^T)
  den [1, q]    = ones^T @ P^T       (lhsT = ones,      rhs = p^T)

Host-side folds: gamma*sqrt(C) into wq/wk/wv; bv through wp into the output
bias (softmax rows sum to 1); no max-subtraction in the softmax (scores here
are O(1); exp is safe and matches jax.nn.softmax exactly up to rounding).

All matmuls run in float32r (FP22) — full-rate on TRN2 with ~1e-4 accuracy.
"""

import sys

import numpy as np

sys.path.insert(0, "/opt/trn_rl_repo")

import concourse.bacc as bacc
import concourse.bass as bass  # noqa: F401
import concourse.tile as tile
from concourse import mybir
from concourse.bass_utils import run_bass_kernel_spmd

C = 512
CC = C // 128          # 4 channel chunks
F = 8                  # frames
HW = 1024              # tokens per frame
SEQ = F * HW           # 8192
S = 512                # kv columns processed per step
KSTEPS = 18            # folded kv half-steps per core (perfectly balanced)
SEQF = KSTEPS * S      # folded kv stream width
Q = 1024               # queries per core (two half-frames: one early, one late)
QH = Q // S            # 2 query halves
SCALE = 1.0 / float(np.sqrt(C))
NEG = -1.0e30

F32 = mybir.dt.float32
F32R = mybir.dt.float32r
Act = mybir.ActivationFunctionType

_cached = {}


def _build():
    if "nc" in _cached:
        return _cached["nc"]

    nc = bacc.Bacc()
    xq_d = nc.dram_tensor("xq", [C, Q], F32, kind="ExternalInput")
    xkv_d = nc.dram_tensor("xkv", [C, SEQF], F32, kind="ExternalInput")
    qoff_d = nc.dram_tensor("qoff", [1, KSTEPS], mybir.dt.int32, kind="ExternalInput")
    wq_d = nc.dram_tensor("wqT", [C, C], F32, kind="ExternalInput")
    wk_d = nc.dram_tensor("wkT", [C, C], F32, kind="ExternalInput")
    wv_d = nc.dram_tensor("wvT", [C, C], F32, kind="ExternalInput")
    wp_d = nc.dram_tensor("wpT", [C, C], F32, kind="ExternalInput")
    bq_d = nc.dram_tensor("bq", [C, 1], F32, kind="ExternalInput")
    bk_d = nc.dram_tensor("bk", [C, 1], F32, kind="ExternalInput")
    bvp_d = nc.dram_tensor("bvp", [C, 1], F32, kind="ExternalInput")
    out_d = nc.dram_tensor("out", [C, Q], F32, kind="ExternalOutput")

    with tile.TileContext(nc) as tc:
        with (
            tc.tile_pool(name="const", bufs=1) as const,
            tc.tile_pool(name="persist", bufs=1) as persist,
            tc.tile_pool(name="xload", bufs=3) as xload,
            tc.tile_pool(name="norm", bufs=2) as norm,
            tc.tile_pool(name="kv", bufs=3) as kvpool,
            tc.tile_pool(name="ppool", bufs=3) as ppool,
            tc.tile_pool(name="dram", bufs=1, space="DRAM") as drampool,
            tc.tile_pool(name="psum_s", bufs=3, space="PSUM") as psum_s,
            tc.tile_pool(name="psum_o", bufs=2, space="PSUM") as psum_o,
            tc.tile_pool(name="psum_den", bufs=1, space="PSUM") as psum_den,
        ):
            # ---- constants / weights (wq and wp share one slot: wp is only
            # needed after the last use of wq) ----
            wq_sb = const.tile([128, CC, C], F32R, tag="wqp", name="wq_sb")
            wk_sb = const.tile([128, CC, C], F32R, tag="wk", name="wk_sb")
            wv_sb = const.tile([128, CC, C], F32R, tag="wv", name="wv_sb")
            for w_sb, w_d in ((wq_sb, wq_d), (wk_sb, wk_d), (wv_sb, wv_d)):
                for ci in range(CC):
                    nc.sync.dma_start(
                        out=w_sb[:, ci, :],
                        in_=w_d[ci * 128:(ci + 1) * 128, :].bitcast(F32R),
                    )
            bq_sb = const.tile([128, CC], F32, tag="bq", name="bq_sb")
            bk_sb = const.tile([128, CC], F32, tag="bk", name="bk_sb")
            bvp_sb = const.tile([128, CC], F32, tag="bvp", name="bvp_sb")
            for b_sb, b_d in ((bq_sb, bq_d), (bk_sb, bk_d), (bvp_sb, bvp_d)):
                for ci in range(CC):
                    nc.sync.dma_start(
                        out=b_sb[:, ci:ci + 1],
                        in_=b_d[ci * 128:(ci + 1) * 128, :],
                    )
            qoff_sb = const.tile([1, KSTEPS], mybir.dt.int32, tag="qoff", name="qoff_sb")
            nc.sync.dma_start(out=qoff_sb[:], in_=qoff_d[:])
            ones_f = const.tile([128, 1], F32, tag="ones_f", name="ones_f")
            nc.vector.memset(ones_f[:], 1.0)
            ones_sb = const.tile([128, 1], F32R, tag="ones", name="ones_sb")
            nc.vector.tensor_copy(ones_sb[:], ones_f[:])
            # PE warmup: ~7us of back-to-back matmuls so the HAM clock gate
            # opens (4/8 -> 8/8) before the real matmul stream begins
            warm_f = norm.tile([128, S], F32, tag="rnb", name="warm_f")
            nc.vector.memset(warm_f[:], 0.0)
            warm_r = norm.tile([128, S], F32R, tag="xsq", name="warm_r")
            nc.vector.tensor_copy(warm_r[:], warm_f[:])
            warm_ps = psum_s.tile([1, S], F32, tag="s", name="warm_ps")
            for wi in range(20):
                nc.tensor.matmul(
                    warm_ps[:], ones_sb[:], warm_r[:],
                    start=(wi == 0), stop=(wi == 19),
                )

            wp_sb = const.tile([128, CC, C], F32R, tag="wqp", name="wp_sb")

            # ---- persistent q-side tiles ----
            qT_sb = persist.tile([128, CC, Q], F32R, tag="qT", name="qT_sb")
            o_sb = persist.tile([128, CC, Q], F32, tag="o", name="o_sb")
            nc.vector.memset(o_sb[:], 0.0)
            rdb = persist.tile([128, Q], F32, tag="rdb", name="rdb")
            den_sb = persist.tile([1, Q], F32, tag="den_sb", name="den_sb")
            nc.vector.memset(den_sb[:], 0.0)

            # ---- single-load pipeline: unit u loads its x slab once,
            # computes column stats (ln of sum x^2 -> DRAM), and one unit
            # later the slab is normalized with the broadcast rnorm and fed
            # to the projections / attention step ----
            UNITS = [(xq_d, qh * S) for qh in range(QH)]
            UNITS += [(xkv_d, t * S) for t in range(KSTEPS)]
            LAG = 1
            xts = {}
            lnts = {}

            def stats_part(u):
                x_dram, col0 = UNITS[u]
                xt = xload.tile([128, CC, S], F32, tag="xt", name="xt")
                xts[u] = xt
                for ci in range(CC):
                    nc.sync.dma_start(
                        out=xt[:, ci, :],
                        in_=x_dram[ci * 128:(ci + 1) * 128, col0:col0 + S],
                    )
                ss_ps = psum_den.tile([1, S], F32, tag="small", name="ss_ps")
                for ci in range(CC):
                    xsq = norm.tile([128, S], F32R, tag="xsq", name="xsq")
                    nc.vector.tensor_mul(xsq[:], xt[:, ci, :], xt[:, ci, :])
                    nc.tensor.matmul(
                        ss_ps[:], ones_sb[:], xsq[:],
                        start=(ci == 0), stop=(ci == CC - 1),
                    )
                ln_t = norm.tile([1, S], F32, tag="ln_t", name="ln_t", bufs=3)
                nc.scalar.activation(ln_t[:], ss_ps[:], Act.Ln)
                lnts[u] = ln_t

            def norm_slab(u):
                lnb = norm.tile([128, S], F32, tag="lnb", name="lnb")
                nc.gpsimd.partition_broadcast(lnb[:], lnts.pop(u)[:])
                nc.scalar.activation(lnb[:], lnb[:], Act.Exp, scale=-0.5)
                xn = norm.tile([128, CC, S], F32R, tag="xn", name="xn")
                xt = xts.pop(u)
                for ci in range(CC):
                    nc.vector.tensor_mul(xn[:, ci, :], xt[:, ci, :], lnb[:])
                return xn

            def work_part(u):
                if u < QH:
                    qh = u
                    xn = norm_slab(u)
                    for co in range(CC):
                        q_ps = psum_o.tile([128, S], F32, tag="proj", name="q_ps")
                        for ci in range(CC):
                            nc.tensor.matmul(
                                q_ps[:],
                                wq_sb[:, ci, co * 128:(co + 1) * 128],
                                xn[:, ci, :],
                                start=(ci == 0), stop=(ci == CC - 1),
                            )
                        nc.vector.tensor_scalar_add(
                            qT_sb[:, co, qh * S:(qh + 1) * S], q_ps[:],
                            bq_sb[:, co:co + 1],
                        )
                    if u == QH - 1:
                        # wp loads into wq's slot once wq is no longer needed
                        for ci in range(CC):
                            nc.sync.dma_start(
                                out=wp_sb[:, ci, :],
                                in_=wp_d[ci * 128:(ci + 1) * 128, :].bitcast(F32R),
                            )
                    return
                t = u - QH
                xn = norm_slab(u)
                off = nc.values_load(
                    qoff_sb[0:1, t:t + 1],
                    engines=[mybir.EngineType.PE, mybir.EngineType.DVE],
                    min_val=0, max_val=S,
                    skip_runtime_bounds_check=True,
                )

                kT = kvpool.tile([128, CC, S], F32R, tag="kT", name="kT")
                for co in range(CC):
                    k_ps = psum_o.tile([128, S], F32, tag="proj", name="k_ps")
                    for ci in range(CC):
                        nc.tensor.matmul(
                            k_ps[:],
                            wk_sb[:, ci, co * 128:(co + 1) * 128],
                            xn[:, ci, :],
                            start=(ci == 0), stop=(ci == CC - 1),
                        )
                    nc.vector.tensor_scalar_add(
                        kT[:, co, :], k_ps[:], bk_sb[:, co:co + 1],
                    )

                v_sb = kvpool.tile([128, S // 128, C], F32R, tag="v", name="v_sb")
                for kp in range(S // 128):
                    v_ps = psum_o.tile([128, C], F32, tag="proj", name="v_ps")
                    for ci in range(CC):
                        nc.tensor.matmul(
                            v_ps[:],
                            xn[:, ci, kp * 128:(kp + 1) * 128],
                            wv_sb[:, ci, :],
                            start=(ci == 0), stop=(ci == CC - 1),
                        )
                    nc.vector.tensor_copy(v_sb[:, kp, :], v_ps[:])

                p_sb = ppool.tile([128, S // 128, S], F32R, tag="p", name="p_sb")
                for kp in range(S // 128):
                    s_ps = psum_s.tile([128, S], F32, tag="s", name="s_ps")
                    for ci in range(CC):
                        nc.tensor.matmul(
                            s_ps[:],
                            kT[:, ci, kp * 128:(kp + 1) * 128],
                            qT_sb[:, ci, bass.ds(off, S)],
                            start=(ci == 0), stop=(ci == CC - 1),
                        )
                    nc.scalar.activation(
                        p_sb[:, kp, :], s_ps[:], Act.Exp, bias=0.0, scale=SCALE,
                    )

                dn_ps = psum_den.tile([1, S], F32, tag="small", name="dn_ps")
                for kp in range(S // 128):
                    nc.tensor.matmul(
                        dn_ps[:], ones_sb[:], p_sb[:, kp, :],
                        start=(kp == 0), stop=(kp == S // 128 - 1),
                    )
                nc.vector.tensor_add(
                    den_sb[:, bass.ds(off, S)],
                    den_sb[:, bass.ds(off, S)],
                    dn_ps[:],
                )

                for co in range(CC):
                    o_ps = psum_o.tile([128, S], F32, tag="o", name="o_ps")
                    for kp in range(S // 128):
                        nc.tensor.matmul(
                            o_ps[:],
                            v_sb[:, kp, co * 128:(co + 1) * 128],
                            p_sb[:, kp, :],
                            start=(kp == 0), stop=(kp == S // 128 - 1),
                        )
                    nc.vector.tensor_add(
                        o_sb[:, co, bass.ds(off, S)],
                        o_sb[:, co, bass.ds(off, S)],
                        o_ps[:],
                    )

            NU = len(UNITS)
            for i in range(NU + LAG):
                if i < NU:
                    stats_part(i)
                if i >= LAG:
                    work_part(i - LAG)

            # ---- finalize: normalize, project, residual ----
            for qh in range(QH):
                rd = norm.tile([1, S], F32, tag="rn", name="rd")
                nc.vector.reciprocal(rd[:], den_sb[:, qh * S:(qh + 1) * S])
                nc.gpsimd.partition_broadcast(rdb[:, qh * S:(qh + 1) * S], rd[:])
            for qh in range(QH):
                # o_n := o * (1/den) for this half, rounded to fp32r
                on_sb = ppool.tile([128, CC, S], F32R, tag="on", name="on_sb", bufs=1)
                for ci in range(CC):
                    nc.vector.tensor_mul(
                        on_sb[:, ci, :], o_sb[:, ci, qh * S:(qh + 1) * S],
                        rdb[:, qh * S:(qh + 1) * S],
                    )
                xr = xload.tile([128, CC, S], F32, tag="xt", name="xr")
                for ci in range(CC):
                    nc.sync.dma_start(
                        out=xr[:, ci, :],
                        in_=xq_d[ci * 128:(ci + 1) * 128, qh * S:(qh + 1) * S],
                    )
                for co in range(CC):
                    pr_ps = psum_o.tile([128, S], F32, tag="proj", name="pr_ps")
                    for ci in range(CC):
                        nc.tensor.matmul(
                            pr_ps[:],
                            wp_sb[:, ci, co * 128:(co + 1) * 128],
                            on_sb[:, ci, :],
                            start=(ci == 0), stop=(ci == CC - 1),
                        )
                    res = norm.tile([128, S], F32, tag="rnb", name="res")
                    nc.vector.scalar_tensor_tensor(
                        out=res[:],
                        in0=pr_ps[:],
                        scalar=bvp_sb[:, co:co + 1],
                        in1=xr[:, co, :],
                        op0=mybir.AluOpType.add,
                        op1=mybir.AluOpType.add,
                    )
                    nc.sync.dma_start(
                        out=out_d[co * 128:(co + 1) * 128, qh * S:(qh + 1) * S],
                        in_=res[:],
                    )

    nc.finalize()
    _cached["nc"] = nc
    return nc


def _prep_inputs(x, gamma, wq, bq, wk, bk, wv, bv, wp, bp):
    x = np.asarray(x, np.float32)
    X = np.ascontiguousarray(x[0].reshape(C, SEQ))
    g = (np.asarray(gamma, np.float32) * np.float32(np.sqrt(C))).astype(np.float32)
    wq = np.asarray(wq, np.float32)
    wk = np.asarray(wk, np.float32)
    wv = np.asarray(wv, np.float32)
    wp = np.asarray(wp, np.float32)
    bq = np.asarray(bq, np.float32)
    bk = np.asarray(bk, np.float32)
    bv = np.asarray(bv, np.float32)
    bp = np.asarray(bp, np.float32)
    wqT = np.ascontiguousarray((wq * g[None, :]).T)
    wkT = np.ascontiguousarray((wk * g[None, :]).T)
    wvT = np.ascontiguousarray((wv * g[None, :]).T)
    wpT = np.ascontiguousarray(wp.T)
    bvp = (bp + wp @ bv).astype(np.float32)

    common = {
        "wqT": wqT, "wkT": wkT, "wvT": wvT, "wpT": wpT,
        "bq": np.ascontiguousarray(bq[:, None]),
        "bk": np.ascontiguousarray(bk[:, None]),
        "bvp": np.ascontiguousarray(bvp[:, None]),
    }
    in_maps = []
    for j in range(F):
        p, half = j // 2, j % 2
        fa, fb = p, F - 1 - p
        qa = X[:, fa * HW + half * S: fa * HW + half * S + S]
        qb = X[:, fb * HW + half * S: fb * HW + half * S + S]
        na, nb = 2 * (fa + 1), 2 * (fb + 1)
        assert na + nb == KSTEPS
        cols = []
        for hf in range(na):
            cols.append(X[:, hf * S:(hf + 1) * S])
        for hf in range(nb):
            cols.append(X[:, hf * S:(hf + 1) * S])
        m = dict(common)
        m["xq"] = np.ascontiguousarray(np.concatenate([qa, qb], axis=1))
        m["xkv"] = np.ascontiguousarray(np.concatenate(cols, axis=1))
        m["qoff"] = np.asarray(
            [[0] * na + [S] * nb], np.int32
        )
        in_maps.append(m)
    return in_maps


def kernel(x, gamma, wq, bq, wk, bk, wv, bv, wp, bp, _trace=False):
    nc = _build()
    in_maps = _prep_inputs(x, gamma, wq, bq, wk, bk, wv, bv, wp, bp)
    kwargs = {}
    if _trace:
        kwargs = dict(trace=True, trace_cores=list(range(F)))
    r = run_bass_kernel_spmd(nc, in_maps, core_ids=list(range(F)), **kwargs)
    out = np.empty((1, C, F, HW), np.float32)
    for j in range(F):
        p, half = j // 2, j % 2
        fa, fb = p, F - 1 - p
        res = r.results[j]["out"]
        out[0, :, fa, half * S:half * S + S] = res[:, 0:S]
        out[0, :, fb, half * S:half * S + S] = res[:, S:Q]
    out = out.reshape(1, C, F, 32, 32)
    kernel._last_results = r
    return out



# revision 2
# speedup vs baseline: 1.2234x; 1.2234x over previous
"""Block-causal attention block (RMSnorm + QKV + frame-causal attention + proj)
on 8 TRN2 NeuronCores — fp8e4 DoubleRow, v8.

Sharding: sequence-parallel over the 8 frames — core j owns 512 queries of
frame p=j//2 (half j%2) and 512 of frame 7-p, and streams the 18 causal kv
half-blocks (512 tokens each) those two query halves attend to.  A per-pair
qoff input steers scores/O into the right query half (one uniform SPMD
program for all cores).

v3 structure (all heavy matmuls fp8e4 DoubleRow, K=256/matmul):
- x arrives pre-quantized to fp8 from the host; the residual re-loads fp32.
- kv tokens are never normalized explicitly: K is cast with a constant
  1/sqrt(C) scale, and the exact per-token 1/||x|| lands in the exp scale
  (per-partition AP) and the V psum->fp8 cast scale.  Only the 2 query slabs
  get a broadcast-multiply normalization.
- per-token rsqrt(sum x^2) is computed with the bit-trick + one Newton step
  on [128,4]-shaped tiles (transposed via 4 tiny SBUF DMAs) — no activation
  tables, so the scalar engine only ever loads the Exp table.
- O/den accumulate in PSUM across each query pair (start/stop spanning two
  kv steps, groups interleaved across banks) halving the SBUF accumulations.
- engine split: PE matmuls; gpsimd squares x (no PSUM port, SBUF-only ops);
  scalar does exp and the V casts; DVE does K/q casts, rsqrt, O/den drains.

Host-side folds: gamma*sqrt(C) and x32 into wq/wk/wv, x32 into wp, bv
through wp into the output bias, bk dropped (cancels in softmax).
Accuracy vs fp32 reference ~4e-4 (tolerance 2e-2).
"""

import sys

import numpy as np

sys.path.insert(0, "/opt/trn_rl_repo")

import ml_dtypes

import concourse.bacc as bacc
import concourse.bass as bass  # noqa: F401
import concourse.tile as tile
from concourse import mybir
from concourse.bass_utils import run_bass_kernel_spmd

C = 512
CC = C // 128          # 4 channel chunks of 128
KK = 2                 # 2 DoubleRow contraction chunks of 256
F = 8                  # frames
HW = 1024              # tokens per frame
SEQ = F * HW           # 8192
S = 512                # kv columns processed per step
KSTEPS = 18            # kv half-steps per core (perfectly balanced)
NPAIRS = KSTEPS // 2
SEQF = KSTEPS * S      # folded kv stream width
Q = 1024               # queries per core (two halves: one early, one late frame)
QH = Q // S            # 2 query halves
WS = 32.0              # fp8 range scale folded into wq/wk/wv/wp
SCALE = 1.0 / float(np.sqrt(C))
KD = 1.0 / float(np.sqrt(C))        # constant K-cast descale (nominal 1/||x||)
EXPSCALE = SCALE / (WS * WS)
RSQE_C = EXPSCALE * float(np.sqrt(C))  # exp scale = RSQE_C * rsq(token)
DESCALE = 1.0 / (WS * WS)
QUAKE_C = 0x5F3759DF
DYN_PAIRS = (1, 2, 3)

F32 = mybir.dt.float32
F8 = mybir.dt.float8e4
I32 = mybir.dt.int32
E4 = ml_dtypes.float8_e4m3
DR = mybir.MatmulPerfMode.DoubleRow
Act = mybir.ActivationFunctionType
Alu = mybir.AluOpType

_cached = {}


def _build():
    if "nc" in _cached:
        return _cached["nc"]

    nc = bacc.Bacc()
    xq_d = nc.dram_tensor("xq8", [128, QH * CC * S], F8, kind="ExternalInput")
    xkv_d = nc.dram_tensor("xkv8", [128, KSTEPS * CC * S], F8, kind="ExternalInput")
    xres_d = nc.dram_tensor("xres", [128, QH * CC * S], F32, kind="ExternalInput")
    qoff_d = nc.dram_tensor("qoff", [1, NPAIRS], I32, kind="ExternalInput")
    wq_d = nc.dram_tensor("wq8", [128, KK * 2 * C], F8, kind="ExternalInput")
    wk_d = nc.dram_tensor("wk8", [128, KK * 2 * C], F8, kind="ExternalInput")
    wv_d = nc.dram_tensor("wv8", [128, KK * 2 * C], F8, kind="ExternalInput")
    wp_d = nc.dram_tensor("wp8", [128, KK * 2 * C], F8, kind="ExternalInput")
    bq_d = nc.dram_tensor("bq", [C, 1], F32, kind="ExternalInput")
    bvp_d = nc.dram_tensor("bvp", [C, 1], F32, kind="ExternalInput")
    out_d = nc.dram_tensor("out", [C, Q], F32, kind="ExternalOutput")

    with tile.TileContext(nc) as tc:
        with (
            tc.tile_pool(name="const", bufs=1) as const,
            tc.tile_pool(name="persist", bufs=1) as persist,
            tc.tile_pool(name="xload", bufs=6) as xload,
            tc.tile_pool(name="norm", bufs=2) as norm,
            tc.tile_pool(name="kv", bufs=3) as kvpool,
            tc.tile_pool(name="ppool", bufs=3) as ppool,
            tc.tile_pool(name="psum_mm", bufs=3, space="PSUM") as psum_mm,
            tc.tile_pool(name="psum_o", bufs=4, space="PSUM") as psum_o,
            tc.tile_pool(name="psum_den", bufs=1, space="PSUM") as psum_den,
        ):
            NU_ALL = QH + KSTEPS
            # ---- prefetch ALL x slabs upfront (40KB/partition): the sync
            # queue then never delays a load behind the per-unit transpose
            # DMAs, so the gpsimd x^2 chain always has its input early ----
            xpre = {}
            for u0 in range(QH):
                xt0 = xload.tile([128, CC, S], F8, tag="xt", name="xt", bufs=NU_ALL)
                nc.sync.dma_start(
                    out=xt0[:], in_=xq_d[:, u0 * CC * S:(u0 + 1) * CC * S],
                )
                xpre[u0] = xt0
            for u0 in range(QH, NU_ALL):
                xt0 = xload.tile([128, CC, S], F8, tag="xt", name="xt", bufs=NU_ALL)
                nc.sync.dma_start(
                    out=xt0[:],
                    in_=xkv_d[:, (u0 - QH) * CC * S:(u0 - QH + 1) * CC * S],
                )
                xpre[u0] = xt0

            # ---- constants / weights ----
            wq_sb = const.tile([128, KK, 2, CC, 128], F8, tag="wq", name="wq_sb")
            wk_sb = const.tile([128, KK, 2, CC, 128], F8, tag="wk", name="wk_sb")
            wv_sb = const.tile([128, KK, 2, CC, 128], F8, tag="wv", name="wv_sb")
            wp_sb = const.tile([128, KK, 2, CC, 128], F8, tag="wp", name="wp_sb")
            for w_sb, w_d in (
                (wq_sb, wq_d), (wk_sb, wk_d), (wv_sb, wv_d), (wp_sb, wp_d),
            ):
                nc.sync.dma_start(out=w_sb[:], in_=w_d[:])
            bq_sb = const.tile([128, CC], F32, tag="bq", name="bq_sb")
            bvp_sb = const.tile([128, CC], F32, tag="bvp", name="bvp_sb")
            for b_sb, b_d in ((bq_sb, bq_d), (bvp_sb, bvp_d)):
                for ci in range(CC):
                    nc.sync.dma_start(
                        out=b_sb[:, ci:ci + 1],
                        in_=b_d[ci * 128:(ci + 1) * 128, :],
                    )
            qoff_sb = const.tile([1, NPAIRS], I32, tag="qoff", name="qoff_sb")
            nc.sync.dma_start(out=qoff_sb[:], in_=qoff_d[:])
            # DR "ones" stationary: [128, 2, 128] with ones in column m=0 only,
            # so ones-reductions land in psum partition 0
            ones_f = const.tile([128, 2, 128], F32, tag="ones_f", name="ones_f")
            nc.vector.memset(ones_f[:], 0.0)
            nc.vector.memset(ones_f[:, :, 0:1], 1.0)
            ones8 = const.tile([128, 2, 128], F8, tag="ones8", name="ones8")
            nc.vector.tensor_copy(ones8[:], ones_f[:])

            # PE warmup: back-to-back fp8 DR matmuls so the HAM clock gate
            # opens (4/8 -> 8/8) before the real matmul stream begins
            warm_f = norm.tile([128, 2, S], F32, tag="lnb", name="warm_f")
            nc.vector.memset(warm_f[:], 0.0)
            warm_r = norm.tile([128, 2, S], F8, tag="warm8", name="warm_r")
            nc.vector.tensor_copy(warm_r[:], warm_f[:])
            warm_ps = psum_mm.tile([128, S], F32, tag="mm", name="warm_ps")
            for wi in range(24):
                nc.tensor.matmul(
                    warm_ps[:], ones8[:], warm_r[:],
                    start=(wi == 0), stop=(wi == 23), perf_mode=DR,
                )

            # ---- persistent q-side tiles ----
            q8_sb = persist.tile([128, CC, Q], F8, tag="qT", name="q8_sb")
            o_sb = persist.tile([128, CC, Q], F32, tag="o", name="o_sb")
            nc.vector.memset(o_sb[:], 0.0)
            rdb = persist.tile([128, Q], F32, tag="rdb", name="rdb")
            den_sb = persist.tile([1, Q], F32, tag="den_sb", name="den_sb")
            nc.vector.memset(den_sb[:], 0.0)

            def quake(out_f32, src_f32, shape, tagp):
                """rsqrt via bit trick + 1 Newton step; all DVE, no tables."""
                ti = norm.tile(shape, I32, tag=tagp + "i", name="q_ti")
                nc.vector.tensor_scalar(
                    out=ti[:], in0=src_f32.bitcast(I32), scalar1=1, scalar2=None,
                    op0=Alu.logical_shift_right,
                )
                nc.vector.tensor_scalar(
                    out=ti[:], in0=ti[:], scalar1=-1, scalar2=QUAKE_C,
                    op0=Alu.mult, op1=Alu.add,
                )
                y0 = ti[:].bitcast(F32)
                h = norm.tile(shape, F32, tag=tagp + "h", name="q_h")
                nc.vector.tensor_mul(h[:], src_f32, y0)
                nc.vector.tensor_mul(h[:], h[:], y0)
                nc.vector.tensor_scalar(
                    out=h[:], in0=h[:], scalar1=-0.5, scalar2=1.5,
                    op0=Alu.mult, op1=Alu.add,
                )
                nc.vector.tensor_mul(out_f32, y0, h[:])

            # ---- single-load pipeline, LAG=2: unit u loads its fp8 x slab,
            # squares it (gpsimd), sums channels on the PE, and derives the
            # per-token 1/||x|| factors; two units later the slab feeds the
            # projections / attention step ----
            UNITS = list(range(NU_ALL))
            LAG = 2
            xts = {}
            rsq1s = {}
            rsqs = {}
            pair_state = {}

            xsqs = {}

            def stats_load(u):
                xt = xpre.pop(u)
                xts[u] = xt
                xsq = norm.tile([128, CC, S], F8, tag="xsq", name="xsq")
                for ci in range(CC):
                    nc.gpsimd.tensor_mul(xsq[:, ci, :], xt[:, ci, :], xt[:, ci, :])
                xsqs[u] = xsq

            def stats_finish(u):
                xsq = xsqs.pop(u)
                ss_ps = psum_mm.tile([128, S], F32, tag="mm", name="ss_ps")
                for k in range(KK):
                    nc.tensor.matmul(
                        ss_ps[:], ones8[:], xsq[:, 2 * k:2 * k + 2, :],
                        start=(k == 0), stop=(k == KK - 1), perf_mode=DR,
                    )
                if u < QH:
                    rsq1 = norm.tile([1, S], F32, tag="rsq1", name="rsq1", bufs=3)
                    quake(rsq1[:], ss_ps[0:1, :], [1, S], "qq")
                    rsq1s[u] = rsq1
                else:
                    ss_sb = norm.tile([1, S], F32, tag="sscp", name="ss_sb", bufs=3)
                    nc.scalar.copy(ss_sb[:], ss_ps[0:1, :])
                    ssT = norm.tile([128, CC], F32, tag="ssT", name="ssT", bufs=3)
                    for kp in range(CC):
                        nc.sync.dma_start(
                            out=ssT[:, kp:kp + 1],
                            in_=ss_sb[0:1, kp * 128:(kp + 1) * 128],
                        )
                    rsqT = norm.tile([128, CC], F32, tag="rsqT", name="rsqT", bufs=3)
                    quake(rsqT[:], ssT[:], [128, CC], "qk")
                    rsqE = norm.tile([128, CC], F32, tag="rsqE", name="rsqE", bufs=3)
                    nc.vector.tensor_scalar_mul(rsqE[:], rsqT[:], RSQE_C)
                    rsqs[u] = (rsqT, rsqE)

            def work_part(u, hook):
                if u < QH:
                    qh = u
                    # normalize the query slab and project
                    rsqb = norm.tile([128, S], F32, tag="lnb", name="rsqb")
                    nc.gpsimd.partition_broadcast(rsqb[:], rsq1s.pop(u)[:])
                    xn = norm.tile([128, CC, S], F8, tag="xn", name="xn")
                    xt = xts.pop(u)
                    for ci in range(CC):
                        nc.vector.tensor_mul(xn[:, ci, :], xt[:, ci, :], rsqb[:])
                    for co in range(CC):
                        q_ps = psum_mm.tile([128, S], F32, tag="mm", name="q_ps")
                        for k in range(KK):
                            nc.tensor.matmul(
                                q_ps[:],
                                wq_sb[:, k, :, co, :],
                                xn[:, 2 * k:2 * k + 2, :],
                                start=(k == 0), stop=(k == KK - 1), perf_mode=DR,
                            )
                        nc.vector.tensor_scalar_add(
                            q8_sb[:, co, qh * S:(qh + 1) * S], q_ps[:],
                            bq_sb[:, co:co + 1],
                        )
                    hook()
                    return

                t = u - QH
                r, phase = divmod(t, 2)
                xt = xts.pop(u)
                rsqT, rsqE = rsqs.pop(u)
                # Query half per pair: with na in {2,4,6,8}, pair 0 always
                # targets the early half and pairs 4+ the late half; only
                # pairs 1-3 vary per core.  Static pairs read q8_sb directly
                # (legal strided AP); dynamic pairs use the q8cur staged by
                # the previous pair (prefetched before the PSUM drains).
                dynamic = r in DYN_PAIRS
                if phase == 0:
                    if dynamic:
                        off, q8cur = pair_state.pop(r)
                        qsrc, qbase = q8cur, 0
                    else:
                        off, qsrc = None, q8_sb
                        qbase = 0 if r == 0 else S
                    dn_ps = psum_den.tile([128, S], F32, tag="den", name="dn_ps")
                    o_pss = [
                        psum_o.tile([128, S], F32, tag="o", name="o_ps")
                        for _ in range(CC)
                    ]
                    pair_state.update(off=off, qsrc=qsrc, qbase=qbase,
                                      dn=dn_ps, o=o_pss)
                else:
                    off = pair_state["off"]
                    qsrc = pair_state["qsrc"]
                    qbase = pair_state["qbase"]
                    dn_ps = pair_state["dn"]
                    o_pss = pair_state["o"]

                # k^T projection on raw fp8 x; constant 1/sqrt(C) range cast
                kT = kvpool.tile([128, CC, S], F8, tag="kT", name="kT")
                for co in range(CC):
                    k_ps = psum_mm.tile([128, S], F32, tag="mm", name="k_ps")
                    for k in range(KK):
                        nc.tensor.matmul(
                            k_ps[:],
                            wk_sb[:, k, :, co, :],
                            xt[:, 2 * k:2 * k + 2, :],
                            start=(k == 0), stop=(k == KK - 1), perf_mode=DR,
                        )
                    nc.scalar.activation(
                        kT[:, co, :], k_ps[:], Act.Copy, scale=KD,
                    )

                # v projection on raw fp8 x; exact per-token 1/||x|| cast scale
                v_sb = kvpool.tile([128, S // 128, C], F8, tag="v", name="v_sb")
                for kp in range(S // 128):
                    v_ps = psum_mm.tile([128, C], F32, tag="mm", name="v_ps")
                    for k in range(KK):
                        nc.tensor.matmul(
                            v_ps[:],
                            xt[:, 2 * k:2 * k + 2, kp * 128:(kp + 1) * 128],
                            wv_sb[:, k, :, :, :],
                            start=(k == 0), stop=(k == KK - 1), perf_mode=DR,
                        )
                    nc.scalar.activation(
                        v_sb[:, kp, :], v_ps[:], Act.Copy,
                        scale=rsqT[:, kp:kp + 1],
                    )

                # scores S^T = K Q^T; P = exp(S^T * scale(token)) in fp8
                p_sb = ppool.tile([128, S // 128, S], F8, tag="p", name="p_sb")
                for kp in range(S // 128):
                    s_ps = psum_mm.tile([128, S], F32, tag="mm", name="s_ps")
                    for k in range(KK):
                        nc.tensor.matmul(
                            s_ps[:],
                            kT[:, 2 * k:2 * k + 2, kp * 128:(kp + 1) * 128],
                            qsrc[:, 2 * k:2 * k + 2, qbase:qbase + S],
                            start=(k == 0), stop=(k == KK - 1), perf_mode=DR,
                        )
                    nc.scalar.activation(
                        p_sb[:, kp, :], s_ps[:], Act.Exp, bias=0.0,
                        scale=rsqE[:, kp:kp + 1],
                    )

                hook()

                # den and O accumulate in PSUM across the pair; groups
                # interleave across banks (hence skip_group_check)
                for k in range(KK):
                    nc.tensor.matmul(
                        dn_ps[:], ones8[:], p_sb[:, 2 * k:2 * k + 2, :],
                        start=(phase == 0 and k == 0),
                        stop=(phase == 1 and k == KK - 1),
                        perf_mode=DR, skip_group_check=True,
                    )
                    for co in range(CC):
                        nc.tensor.matmul(
                            o_pss[co][:],
                            v_sb[:, 2 * k:2 * k + 2, co * 128:(co + 1) * 128],
                            p_sb[:, 2 * k:2 * k + 2, :],
                            start=(phase == 0 and k == 0),
                            stop=(phase == 1 and k == KK - 1),
                            perf_mode=DR, skip_group_check=True,
                        )
                if phase == 1:
                    if (r + 1) in DYN_PAIRS:
                        noff = nc.values_load(
                            qoff_sb[0:1, r + 1:r + 2],
                            engines=[mybir.EngineType.DVE],
                            min_val=0, max_val=S,
                            skip_runtime_bounds_check=True,
                        )
                        q8n = kvpool.tile(
                            [128, CC, S], F8, tag="qcur", name="q8cur", bufs=2,
                        )
                        for ci in range(CC):
                            nc.vector.tensor_copy(
                                q8n[:, ci, :], q8_sb[:, ci, bass.ds(noff, S)],
                            )
                        pair_state[r + 1] = (noff, q8n)
                    if off is None:
                        nc.vector.tensor_add(
                            den_sb[:, qbase:qbase + S],
                            den_sb[:, qbase:qbase + S],
                            dn_ps[0:1, :],
                        )
                        for co in range(CC):
                            nc.vector.tensor_add(
                                o_sb[:, co, qbase:qbase + S],
                                o_sb[:, co, qbase:qbase + S],
                                o_pss[co][:],
                            )
                    else:
                        nc.vector.tensor_add(
                            den_sb[:, bass.ds(off, S)],
                            den_sb[:, bass.ds(off, S)],
                            dn_ps[0:1, :],
                        )
                        for co in range(CC):
                            nc.vector.tensor_add(
                                o_sb[:, co, bass.ds(off, S)],
                                o_sb[:, co, bass.ds(off, S)],
                                o_pss[co][:],
                            )

            NU = len(UNITS)
            for i in range(NU + LAG):
                if i < NU:
                    stats_load(i)
                if i < LAG:
                    stats_finish(i)
                    if i == LAG - 1:
                        warm_ps2 = psum_mm.tile([128, S], F32, tag="mm", name="warm_ps2")
                        for wi in range(20):
                            nc.tensor.matmul(
                                warm_ps2[:], ones8[:], warm_r[:],
                                start=(wi == 0), stop=(wi == 19), perf_mode=DR,
                            )
                else:
                    hook = (lambda i=i: stats_finish(i)) if i < NU else (lambda: None)
                    work_part(i - LAG, hook)

            # ---- finalize: normalize, project, residual ----
            for qh in range(QH):
                rd = norm.tile([1, S], F32, tag="rn", name="rd")
                nc.vector.reciprocal_approx_fast(
                    out=rd[:], in_=den_sb[:, qh * S:(qh + 1) * S],
                )
                nc.gpsimd.partition_broadcast(rdb[:, qh * S:(qh + 1) * S], rd[:])
            for qh in range(QH):
                # o_n := o * (1/den) for this half, cast to fp8
                on_sb = ppool.tile([128, CC, S], F8, tag="on", name="on_sb", bufs=2)
                for ci in range(CC):
                    nc.vector.tensor_mul(
                        on_sb[:, ci, :], o_sb[:, ci, qh * S:(qh + 1) * S],
                        rdb[:, qh * S:(qh + 1) * S],
                    )
                xr = xload.tile([128, CC, S], F32, tag="xr", name="xr", bufs=2)
                nc.sync.dma_start(
                    out=xr[:],
                    in_=xres_d[:, qh * CC * S:(qh + 1) * CC * S],
                )
                for co in range(CC):
                    pr_ps = psum_mm.tile([128, S], F32, tag="mm", name="pr_ps")
                    for k in range(KK):
                        nc.tensor.matmul(
                            pr_ps[:],
                            wp_sb[:, k, :, co, :],
                            on_sb[:, 2 * k:2 * k + 2, :],
                            start=(k == 0), stop=(k == KK - 1), perf_mode=DR,
                        )
                    prs = norm.tile([128, S], F32, tag="prs", name="prs")
                    nc.scalar.mul(prs[:], pr_ps[:], DESCALE)
                    res = norm.tile([128, S], F32, tag="res", name="res")
                    nc.vector.scalar_tensor_tensor(
                        out=res[:],
                        in0=prs[:],
                        scalar=bvp_sb[:, co:co + 1],
                        in1=xr[:, co, :],
                        op0=Alu.add,
                        op1=Alu.add,
                    )
                    nc.sync.dma_start(
                        out=out_d[co * 128:(co + 1) * 128, qh * S:(qh + 1) * S],
                        in_=res[:],
                    )

    nc.finalize()
    _cached["nc"] = nc
    return nc


def _dr_layout(wt):
    """[C_in, C_out] f32 -> [128, KK*2*C_out] fp8 in DoubleRow stationary
    order: [p, k, i, co, m] = wt[k*256 + i*128 + p, co*128 + m]."""
    t = wt.reshape(KK, 2, 128, CC, 128).transpose(2, 0, 1, 3, 4)
    return np.ascontiguousarray(t.reshape(128, KK * 2 * C)).astype(E4)


def _swizzle(xcs):
    """[C, n*S] -> [128, n*CC*S]: slab n contiguous as [CC, S] per partition."""
    n = xcs.shape[1] // S
    t = xcs.reshape(CC, 128, n, S).transpose(1, 2, 0, 3)
    return np.ascontiguousarray(t.reshape(128, n * CC * S))


def _prep_inputs(x, gamma, wq, bq, wk, bk, wv, bv, wp, bp):
    x = np.asarray(x, np.float32)
    X = np.ascontiguousarray(x[0].reshape(C, SEQ))
    X8 = X.astype(E4)
    g = (np.asarray(gamma, np.float32) * np.float32(np.sqrt(C))).astype(np.float32)
    wq = np.asarray(wq, np.float32)
    wk = np.asarray(wk, np.float32)
    wv = np.asarray(wv, np.float32)
    wp = np.asarray(wp, np.float32)
    bq = np.asarray(bq, np.float32)
    bv = np.asarray(bv, np.float32)
    bp = np.asarray(bp, np.float32)
    wq8 = _dr_layout((wq * g[None, :]).T * WS)
    wk8 = _dr_layout((wk * g[None, :]).T * WS)
    wv8 = _dr_layout((wv * g[None, :]).T * WS)
    wp8 = _dr_layout(wp.T * WS)
    bvp = (bp + wp @ bv).astype(np.float32)

    common = {
        "wq8": wq8, "wk8": wk8, "wv8": wv8, "wp8": wp8,
        "bq": np.ascontiguousarray((bq * WS)[:, None]).astype(np.float32),
        "bvp": np.ascontiguousarray(bvp[:, None]),
    }
    in_maps = []
    for j in range(F):
        p, half = j // 2, j % 2
        fa, fb = p, F - 1 - p
        c0a = fa * HW + half * S
        c0b = fb * HW + half * S
        na, nb = 2 * (fa + 1), 2 * (fb + 1)
        assert na + nb == KSTEPS
        cols = []
        for hf in range(na):
            cols.append(X8[:, hf * S:(hf + 1) * S])
        for hf in range(nb):
            cols.append(X8[:, hf * S:(hf + 1) * S])
        m = dict(common)
        m["xq8"] = _swizzle(
            np.concatenate([X8[:, c0a:c0a + S], X8[:, c0b:c0b + S]], axis=1))
        m["xkv8"] = _swizzle(np.concatenate(cols, axis=1))
        m["xres"] = _swizzle(
            np.concatenate([X[:, c0a:c0a + S], X[:, c0b:c0b + S]], axis=1))
        m["qoff"] = np.asarray(
            [[0] * (na // 2) + [S] * (nb // 2)], np.int32
        )
        in_maps.append(m)
    return in_maps


def kernel(x, gamma, wq, bq, wk, bk, wv, bv, wp, bp, _trace=False):
    nc = _build()
    in_maps = _prep_inputs(x, gamma, wq, bq, wk, bk, wv, bv, wp, bp)
    kwargs = {}
    if _trace:
        kwargs = dict(trace=True, trace_cores=list(range(F)))
    r = run_bass_kernel_spmd(nc, in_maps, core_ids=list(range(F)), **kwargs)
    out = np.empty((1, C, F, HW), np.float32)
    for j in range(F):
        p, half = j // 2, j % 2
        fa, fb = p, F - 1 - p
        res = r.results[j]["out"]
        out[0, :, fa, half * S:half * S + S] = res[:, 0:S]
        out[0, :, fb, half * S:half * S + S] = res[:, S:Q]
    out = out.reshape(1, C, F, 32, 32)
    kernel._last_results = r
    return out


# revision 3
# speedup vs baseline: 1.2306x; 1.0059x over previous
"""Block-causal attention block (RMSnorm + QKV + frame-causal attention + proj)
on 8 TRN2 NeuronCores — fp8e4 DoubleRow, v10.

Sharding: sequence-parallel over the 8 frames — core j owns 512 queries of
frame p=j//2 (half j%2) and 512 of frame 7-p, and streams the 18 causal kv
half-blocks (512 tokens each) those two query halves attend to.  A per-pair
qoff input steers scores/O into the right query half (one uniform SPMD
program for all cores).

v3 structure (all heavy matmuls fp8e4 DoubleRow, K=256/matmul):
- x arrives pre-quantized to fp8 from the host; the residual re-loads fp32.
- kv tokens are never normalized explicitly: K is cast with a constant
  1/sqrt(C) scale, and the exact per-token 1/||x|| lands in the exp scale
  (per-partition AP) and the V psum->fp8 cast scale.  Only the 2 query slabs
  get a broadcast-multiply normalization.
- per-token rsqrt(sum x^2) is computed with the bit-trick + one Newton step
  on [128,4]-shaped tiles (transposed via 4 tiny SBUF DMAs) — no activation
  tables, so the scalar engine only ever loads the Exp table.
- O/den accumulate in PSUM across each query pair (start/stop spanning two
  kv steps, groups interleaved across banks) halving the SBUF accumulations.
- engine split: PE matmuls; gpsimd squares x (no PSUM port, SBUF-only ops);
  scalar does exp and the V casts; DVE does K/q casts, rsqrt, O/den drains.

Host-side folds: gamma*sqrt(C) and x32 into wq/wk/wv, x32 into wp, bv
through wp into the output bias, bk dropped (cancels in softmax).
Accuracy vs fp32 reference ~4e-4 (tolerance 2e-2).
"""

import sys

import numpy as np

sys.path.insert(0, "/opt/trn_rl_repo")

import ml_dtypes

import concourse.bacc as bacc
import concourse.bass as bass  # noqa: F401
import concourse.tile as tile
from concourse import mybir
from concourse.bass_utils import run_bass_kernel_spmd

C = 512
CC = C // 128          # 4 channel chunks of 128
KK = 2                 # 2 DoubleRow contraction chunks of 256
F = 8                  # frames
HW = 1024              # tokens per frame
SEQ = F * HW           # 8192
S = 512                # kv columns processed per step
KSTEPS = 18            # kv half-steps per core (perfectly balanced)
NPAIRS = KSTEPS // 2
SEQF = KSTEPS * S      # folded kv stream width
Q = 1024               # queries per core (two halves: one early, one late frame)
QH = Q // S            # 2 query halves
WS = 32.0              # fp8 range scale folded into wq/wk/wv/wp
SCALE = 1.0 / float(np.sqrt(C))
KD = 1.0 / float(np.sqrt(C))        # constant K-cast descale (nominal 1/||x||)
EXPSCALE = SCALE / (WS * WS)
RSQE_C = EXPSCALE * float(np.sqrt(C))  # exp scale = RSQE_C * rsq(token)
DESCALE = 1.0 / (WS * WS)
QUAKE_C = 0x5F3759DF
DYN_PAIRS = (1, 2, 3)

F32 = mybir.dt.float32
F8 = mybir.dt.float8e4
I32 = mybir.dt.int32
E4 = ml_dtypes.float8_e4m3
DR = mybir.MatmulPerfMode.DoubleRow
Act = mybir.ActivationFunctionType
Alu = mybir.AluOpType

_cached = {}


def _build():
    if "nc" in _cached:
        return _cached["nc"]

    nc = bacc.Bacc()
    xq_d = nc.dram_tensor("xq8", [128, QH * CC * S], F8, kind="ExternalInput")
    xkv_d = nc.dram_tensor("xkv8", [128, KSTEPS * CC * S], F8, kind="ExternalInput")
    xres_d = nc.dram_tensor("xres", [128, QH * CC * S], F32, kind="ExternalInput")
    qoff_d = nc.dram_tensor("qoff", [1, NPAIRS], I32, kind="ExternalInput")
    wq_d = nc.dram_tensor("wq8", [128, KK * 2 * C], F8, kind="ExternalInput")
    wk_d = nc.dram_tensor("wk8", [128, KK * 2 * C], F8, kind="ExternalInput")
    wv_d = nc.dram_tensor("wv8", [128, KK * 2 * C], F8, kind="ExternalInput")
    wp_d = nc.dram_tensor("wp8", [128, KK * 2 * C], F8, kind="ExternalInput")
    bq_d = nc.dram_tensor("bq", [C, 1], F32, kind="ExternalInput")
    bvp_d = nc.dram_tensor("bvp", [C, 1], F32, kind="ExternalInput")
    out_d = nc.dram_tensor("out", [C, Q], F32, kind="ExternalOutput")

    with tile.TileContext(nc) as tc:
        with (
            tc.tile_pool(name="const", bufs=1) as const,
            tc.tile_pool(name="persist", bufs=1) as persist,
            tc.tile_pool(name="xload", bufs=6) as xload,
            tc.tile_pool(name="norm", bufs=2) as norm,
            tc.tile_pool(name="kv", bufs=3) as kvpool,
            tc.tile_pool(name="ppool", bufs=3) as ppool,
            tc.tile_pool(name="psum_mm", bufs=3, space="PSUM") as psum_mm,
            tc.tile_pool(name="psum_o", bufs=4, space="PSUM") as psum_o,
            tc.tile_pool(name="psum_den", bufs=1, space="PSUM") as psum_den,
        ):
            NU_ALL = QH + KSTEPS
            # ---- prefetch ALL x slabs upfront (40KB/partition): the sync
            # queue then never delays a load behind the per-unit transpose
            # DMAs, so the gpsimd x^2 chain always has its input early ----
            xpre = {}
            for u0 in range(QH):
                xt0 = xload.tile([128, CC, S], F8, tag="xt", name="xt", bufs=NU_ALL)
                nc.sync.dma_start(
                    out=xt0[:], in_=xq_d[:, u0 * CC * S:(u0 + 1) * CC * S],
                )
                xpre[u0] = xt0
            for u0 in range(QH, NU_ALL):
                xt0 = xload.tile([128, CC, S], F8, tag="xt", name="xt", bufs=NU_ALL)
                nc.sync.dma_start(
                    out=xt0[:],
                    in_=xkv_d[:, (u0 - QH) * CC * S:(u0 - QH + 1) * CC * S],
                )
                xpre[u0] = xt0

            # ---- constants / weights ----
            wq_sb = const.tile([128, KK, 2, CC, 128], F8, tag="wq", name="wq_sb")
            wk_sb = const.tile([128, KK, 2, CC, 128], F8, tag="wk", name="wk_sb")
            wv_sb = const.tile([128, KK, 2, CC, 128], F8, tag="wv", name="wv_sb")
            wp_sb = const.tile([128, KK, 2, CC, 128], F8, tag="wp", name="wp_sb")
            for w_sb, w_d in (
                (wq_sb, wq_d), (wk_sb, wk_d), (wv_sb, wv_d), (wp_sb, wp_d),
            ):
                nc.sync.dma_start(out=w_sb[:], in_=w_d[:])
            bq_sb = const.tile([128, CC], F32, tag="bq", name="bq_sb")
            bvp_sb = const.tile([128, CC], F32, tag="bvp", name="bvp_sb")
            for b_sb, b_d in ((bq_sb, bq_d), (bvp_sb, bvp_d)):
                for ci in range(CC):
                    nc.sync.dma_start(
                        out=b_sb[:, ci:ci + 1],
                        in_=b_d[ci * 128:(ci + 1) * 128, :],
                    )
            qoff_sb = const.tile([1, NPAIRS], I32, tag="qoff", name="qoff_sb")
            nc.sync.dma_start(out=qoff_sb[:], in_=qoff_d[:])
            # DR "ones" stationary: [128, 2, 128] with ones in column m=0 only,
            # so ones-reductions land in psum partition 0
            ones_f = const.tile([128, 2, 128], F32, tag="ones_f", name="ones_f")
            nc.vector.memset(ones_f[:], 0.0)
            nc.vector.memset(ones_f[:, :, 0:1], 1.0)
            ones8 = const.tile([128, 2, 128], F8, tag="ones8", name="ones8")
            nc.vector.tensor_copy(ones8[:], ones_f[:])
            # [1,128] fp32 ones: PE-side partition broadcast via K=1 matmul
            ones1 = const.tile([1, 128], F32, tag="ones1", name="ones1")
            nc.vector.memset(ones1[:], 1.0)

            # PE warmup: back-to-back fp8 DR matmuls so the HAM clock gate
            # opens (4/8 -> 8/8) before the real matmul stream begins
            warm_f = norm.tile([128, 2, S], F32, tag="lnb", name="warm_f")
            nc.vector.memset(warm_f[:], 0.0)
            warm_r = norm.tile([128, 2, S], F8, tag="warm8", name="warm_r")
            nc.vector.tensor_copy(warm_r[:], warm_f[:])
            warm_ps = psum_mm.tile([128, S], F32, tag="mm", name="warm_ps")
            for wi in range(24):
                nc.tensor.matmul(
                    warm_ps[:], ones8[:], warm_r[:],
                    start=(wi == 0), stop=(wi == 23), perf_mode=DR,
                )

            # ---- persistent q-side tiles ----
            q8_sb = persist.tile([128, CC, Q], F8, tag="qT", name="q8_sb")
            o_sb = persist.tile([128, CC, Q], F32, tag="o", name="o_sb")
            nc.vector.memset(o_sb[:], 0.0)
            den_sb = persist.tile([1, Q], F32, tag="den_sb", name="den_sb")
            nc.vector.memset(den_sb[:], 0.0)

            def quake(out_f32, src_f32, shape, tagp):
                """rsqrt via bit trick + 1 Newton step; all DVE, no tables."""
                ti = norm.tile(shape, I32, tag=tagp + "i", name="q_ti")
                nc.vector.tensor_scalar(
                    out=ti[:], in0=src_f32.bitcast(I32), scalar1=1, scalar2=None,
                    op0=Alu.logical_shift_right,
                )
                nc.vector.tensor_scalar(
                    out=ti[:], in0=ti[:], scalar1=-1, scalar2=QUAKE_C,
                    op0=Alu.mult, op1=Alu.add,
                )
                y0 = ti[:].bitcast(F32)
                h = norm.tile(shape, F32, tag=tagp + "h", name="q_h")
                nc.vector.tensor_mul(h[:], src_f32, y0)
                nc.vector.tensor_mul(h[:], h[:], y0)
                nc.vector.tensor_scalar(
                    out=h[:], in0=h[:], scalar1=-0.5, scalar2=1.5,
                    op0=Alu.mult, op1=Alu.add,
                )
                nc.vector.tensor_mul(out_f32, y0, h[:])

            # ---- single-load pipeline, LAG=2: unit u loads its fp8 x slab,
            # squares it (gpsimd), sums channels on the PE, and derives the
            # per-token 1/||x|| factors; two units later the slab feeds the
            # projections / attention step ----
            UNITS = list(range(NU_ALL))
            LAG = 2
            xts = {}
            rsq1s = {}
            rsqs = {}
            pair_state = {}

            xsqs = {}

            def stats_load(u):
                xt = xpre.pop(u)
                xts[u] = xt
                xsq = norm.tile([128, CC, S], F8, tag="xsq", name="xsq")
                for ci in range(CC):
                    nc.gpsimd.tensor_mul(xsq[:, ci, :], xt[:, ci, :], xt[:, ci, :])
                xsqs[u] = xsq

            def stats_finish(u):
                xsq = xsqs.pop(u)
                ss_ps = psum_mm.tile([128, S], F32, tag="mm", name="ss_ps")
                for k in range(KK):
                    nc.tensor.matmul(
                        ss_ps[:], ones8[:], xsq[:, 2 * k:2 * k + 2, :],
                        start=(k == 0), stop=(k == KK - 1), perf_mode=DR,
                    )
                if u < QH:
                    rsq1 = norm.tile([1, S], F32, tag="rsq1", name="rsq1", bufs=3)
                    quake(rsq1[:], ss_ps[0:1, :], [1, S], "qq")
                    rsq1s[u] = rsq1
                else:
                    ss_sb = norm.tile([1, S], F32, tag="sscp", name="ss_sb", bufs=3)
                    nc.scalar.copy(ss_sb[:], ss_ps[0:1, :])
                    ssT = norm.tile([128, CC], F32, tag="ssT", name="ssT", bufs=3)
                    for kp in range(CC):
                        nc.sync.dma_start(
                            out=ssT[:, kp:kp + 1],
                            in_=ss_sb[0:1, kp * 128:(kp + 1) * 128],
                        )
                    rsqT = norm.tile([128, CC], F32, tag="rsqT", name="rsqT", bufs=3)
                    quake(rsqT[:], ssT[:], [128, CC], "qk")
                    rsqE = norm.tile([128, CC], F32, tag="rsqE", name="rsqE", bufs=3)
                    nc.vector.tensor_scalar_mul(rsqE[:], rsqT[:], RSQE_C)
                    rsqs[u] = (rsqT, rsqE)

            def work_part(u, hook):
                if u < QH:
                    qh = u
                    # normalize the query slab and project; the 1/||x|| row is
                    # partition-broadcast by a K=1 fp32 matmul (PE) so the
                    # gpsimd x^2 backlog never gates the ramp
                    rsqb = psum_o.tile([128, S], F32, tag="o", name="rsqb_ps")
                    nc.tensor.matmul(
                        rsqb[:], ones1[:], rsq1s.pop(u)[:], start=True, stop=True,
                    )
                    xn = norm.tile([128, CC, S], F8, tag="xn", name="xn")
                    xt = xts.pop(u)
                    for ci in range(CC):
                        nc.vector.tensor_mul(xn[:, ci, :], xt[:, ci, :], rsqb[:])
                    for co in range(CC):
                        q_ps = psum_mm.tile([128, S], F32, tag="mm", name="q_ps")
                        for k in range(KK):
                            nc.tensor.matmul(
                                q_ps[:],
                                wq_sb[:, k, :, co, :],
                                xn[:, 2 * k:2 * k + 2, :],
                                start=(k == 0), stop=(k == KK - 1), perf_mode=DR,
                            )
                        nc.vector.tensor_scalar_add(
                            q8_sb[:, co, qh * S:(qh + 1) * S], q_ps[:],
                            bq_sb[:, co:co + 1],
                        )
                    hook()
                    if qh == QH - 1:
                        for rr in DYN_PAIRS:
                            roff = nc.values_load(
                                qoff_sb[0:1, rr:rr + 1],
                                engines=[mybir.EngineType.DVE],
                                min_val=0, max_val=S,
                                skip_runtime_bounds_check=True,
                            )
                            q8r = kvpool.tile(
                                [128, CC, S], F8, tag="qcur", name="q8cur",
                                bufs=len(DYN_PAIRS),
                            )
                            for ci in range(CC):
                                nc.vector.tensor_copy(
                                    q8r[:, ci, :],
                                    q8_sb[:, ci, bass.ds(roff, S)],
                                )
                            pair_state[rr] = (roff, q8r)
                    return

                t = u - QH
                r, phase = divmod(t, 2)
                xt = xts.pop(u)
                rsqT, rsqE = rsqs.pop(u)
                # Query half per pair: with na in {2,4,6,8}, pair 0 always
                # targets the early half and pairs 4+ the late half; only
                # pairs 1-3 vary per core.  Static pairs read q8_sb directly
                # (legal strided AP); dynamic pairs use the q8cur staged by
                # the previous pair (prefetched before the PSUM drains).
                dynamic = r in DYN_PAIRS
                if phase == 0:
                    if dynamic:
                        off, q8cur = pair_state.pop(r)
                        qsrc, qbase = q8cur, 0
                    else:
                        off, qsrc = None, q8_sb
                        qbase = 0 if r == 0 else S
                    dn_ps = psum_den.tile([128, S], F32, tag="den", name="dn_ps")
                    o_pss = [
                        psum_o.tile([128, S], F32, tag="o", name="o_ps")
                        for _ in range(CC)
                    ]
                    pair_state.update(off=off, qsrc=qsrc, qbase=qbase,
                                      dn=dn_ps, o=o_pss)
                else:
                    off = pair_state["off"]
                    qsrc = pair_state["qsrc"]
                    qbase = pair_state["qbase"]
                    dn_ps = pair_state["dn"]
                    o_pss = pair_state["o"]

                # k^T projection on raw fp8 x; constant 1/sqrt(C) range cast
                kT = kvpool.tile([128, CC, S], F8, tag="kT", name="kT")
                for co in range(CC):
                    k_ps = psum_mm.tile([128, S], F32, tag="mm", name="k_ps")
                    for k in range(KK):
                        nc.tensor.matmul(
                            k_ps[:],
                            wk_sb[:, k, :, co, :],
                            xt[:, 2 * k:2 * k + 2, :],
                            start=(k == 0), stop=(k == KK - 1), perf_mode=DR,
                        )
                    nc.scalar.activation(
                        kT[:, co, :], k_ps[:], Act.Copy, scale=KD,
                    )

                # v projection on raw fp8 x; exact per-token 1/||x|| cast scale
                v_sb = kvpool.tile([128, S // 128, C], F8, tag="v", name="v_sb")
                for kp in range(S // 128):
                    v_ps = psum_mm.tile([128, C], F32, tag="mm", name="v_ps")
                    for k in range(KK):
                        nc.tensor.matmul(
                            v_ps[:],
                            xt[:, 2 * k:2 * k + 2, kp * 128:(kp + 1) * 128],
                            wv_sb[:, k, :, :, :],
                            start=(k == 0), stop=(k == KK - 1), perf_mode=DR,
                        )
                    nc.scalar.activation(
                        v_sb[:, kp, :], v_ps[:], Act.Copy,
                        scale=rsqT[:, kp:kp + 1],
                    )

                # scores S^T = K Q^T; P = exp(S^T * scale(token)) in fp8
                p_sb = ppool.tile([128, S // 128, S], F8, tag="p", name="p_sb")
                for kp in range(S // 128):
                    s_ps = psum_mm.tile([128, S], F32, tag="mm", name="s_ps")
                    for k in range(KK):
                        nc.tensor.matmul(
                            s_ps[:],
                            kT[:, 2 * k:2 * k + 2, kp * 128:(kp + 1) * 128],
                            qsrc[:, 2 * k:2 * k + 2, qbase:qbase + S],
                            start=(k == 0), stop=(k == KK - 1), perf_mode=DR,
                        )
                    nc.scalar.activation(
                        p_sb[:, kp, :], s_ps[:], Act.Exp, bias=0.0,
                        scale=rsqE[:, kp:kp + 1],
                    )

                hook()

                # den and O accumulate in PSUM across the pair; groups
                # interleave across banks (hence skip_group_check)
                for k in range(KK):
                    nc.tensor.matmul(
                        dn_ps[:], ones8[:], p_sb[:, 2 * k:2 * k + 2, :],
                        start=(phase == 0 and k == 0),
                        stop=(phase == 1 and k == KK - 1),
                        perf_mode=DR, skip_group_check=True,
                    )
                    for co in range(CC):
                        nc.tensor.matmul(
                            o_pss[co][:],
                            v_sb[:, 2 * k:2 * k + 2, co * 128:(co + 1) * 128],
                            p_sb[:, 2 * k:2 * k + 2, :],
                            start=(phase == 0 and k == 0),
                            stop=(phase == 1 and k == KK - 1),
                            perf_mode=DR, skip_group_check=True,
                        )
                if phase == 1:
                    if off is None:
                        nc.vector.tensor_add(
                            den_sb[:, qbase:qbase + S],
                            den_sb[:, qbase:qbase + S],
                            dn_ps[0:1, :],
                        )
                        for co in range(CC):
                            nc.vector.tensor_add(
                                o_sb[:, co, qbase:qbase + S],
                                o_sb[:, co, qbase:qbase + S],
                                o_pss[co][:],
                            )
                    else:
                        nc.vector.tensor_add(
                            den_sb[:, bass.ds(off, S)],
                            den_sb[:, bass.ds(off, S)],
                            dn_ps[0:1, :],
                        )
                        for co in range(CC):
                            nc.vector.tensor_add(
                                o_sb[:, co, bass.ds(off, S)],
                                o_sb[:, co, bass.ds(off, S)],
                                o_pss[co][:],
                            )

            NU = len(UNITS)
            for i in range(NU + LAG):
                if i < NU:
                    stats_load(i)
                if i < LAG:
                    stats_finish(i)
                    if i == LAG - 1:
                        warm_ps2 = psum_mm.tile([128, S], F32, tag="mm", name="warm_ps2")
                        for wi in range(20):
                            nc.tensor.matmul(
                                warm_ps2[:], ones8[:], warm_r[:],
                                start=(wi == 0), stop=(wi == 19), perf_mode=DR,
                            )
                else:
                    hook = (lambda i=i: stats_finish(i)) if i < NU else (lambda: None)
                    work_part(i - LAG, hook)

            # ---- finalize: normalize, project, residual ----
            rdbs = []
            for qh in range(QH):
                rd = norm.tile([1, S], F32, tag="rn", name="rd", bufs=2)
                nc.vector.reciprocal_approx_fast(
                    out=rd[:], in_=den_sb[:, qh * S:(qh + 1) * S],
                )
                rdb_ps = psum_o.tile([128, S], F32, tag="o", name="rdb_ps")
                nc.tensor.matmul(
                    rdb_ps[:], ones1[:], rd[:], start=True, stop=True,
                )
                rdbs.append(rdb_ps)
            for qh in range(QH):
                # o_n := o * (1/den) for this half, cast to fp8
                on_sb = ppool.tile([128, CC, S], F8, tag="on", name="on_sb", bufs=2)
                for ci in range(CC):
                    nc.vector.tensor_mul(
                        on_sb[:, ci, :], o_sb[:, ci, qh * S:(qh + 1) * S],
                        rdbs[qh][:],
                    )
                xr = xload.tile([128, CC, S], F32, tag="xr", name="xr", bufs=2)
                nc.sync.dma_start(
                    out=xr[:],
                    in_=xres_d[:, qh * CC * S:(qh + 1) * CC * S],
                )
                for co in range(CC):
                    pr_ps = psum_mm.tile([128, S], F32, tag="mm", name="pr_ps")
                    for k in range(KK):
                        nc.tensor.matmul(
                            pr_ps[:],
                            wp_sb[:, k, :, co, :],
                            on_sb[:, 2 * k:2 * k + 2, :],
                            start=(k == 0), stop=(k == KK - 1), perf_mode=DR,
                        )
                    prs = norm.tile([128, S], F32, tag="prs", name="prs")
                    nc.scalar.mul(prs[:], pr_ps[:], DESCALE)
                    res = norm.tile([128, S], F32, tag="res", name="res")
                    nc.vector.scalar_tensor_tensor(
                        out=res[:],
                        in0=prs[:],
                        scalar=bvp_sb[:, co:co + 1],
                        in1=xr[:, co, :],
                        op0=Alu.add,
                        op1=Alu.add,
                    )
                    nc.sync.dma_start(
                        out=out_d[co * 128:(co + 1) * 128, qh * S:(qh + 1) * S],
                        in_=res[:],
                    )

    nc.finalize()
    _cached["nc"] = nc
    return nc


def _dr_layout(wt):
    """[C_in, C_out] f32 -> [128, KK*2*C_out] fp8 in DoubleRow stationary
    order: [p, k, i, co, m] = wt[k*256 + i*128 + p, co*128 + m]."""
    t = wt.reshape(KK, 2, 128, CC, 128).transpose(2, 0, 1, 3, 4)
    return np.ascontiguousarray(t.reshape(128, KK * 2 * C)).astype(E4)


def _swizzle(xcs):
    """[C, n*S] -> [128, n*CC*S]: slab n contiguous as [CC, S] per partition."""
    n = xcs.shape[1] // S
    t = xcs.reshape(CC, 128, n, S).transpose(1, 2, 0, 3)
    return np.ascontiguousarray(t.reshape(128, n * CC * S))


def _prep_inputs(x, gamma, wq, bq, wk, bk, wv, bv, wp, bp):
    x = np.asarray(x, np.float32)
    X = np.ascontiguousarray(x[0].reshape(C, SEQ))
    X8 = X.astype(E4)
    g = (np.asarray(gamma, np.float32) * np.float32(np.sqrt(C))).astype(np.float32)
    wq = np.asarray(wq, np.float32)
    wk = np.asarray(wk, np.float32)
    wv = np.asarray(wv, np.float32)
    wp = np.asarray(wp, np.float32)
    bq = np.asarray(bq, np.float32)
    bv = np.asarray(bv, np.float32)
    bp = np.asarray(bp, np.float32)
    wq8 = _dr_layout((wq * g[None, :]).T * WS)
    wk8 = _dr_layout((wk * g[None, :]).T * WS)
    wv8 = _dr_layout((wv * g[None, :]).T * WS)
    wp8 = _dr_layout(wp.T * WS)
    bvp = (bp + wp @ bv).astype(np.float32)

    common = {
        "wq8": wq8, "wk8": wk8, "wv8": wv8, "wp8": wp8,
        "bq": np.ascontiguousarray((bq * WS)[:, None]).astype(np.float32),
        "bvp": np.ascontiguousarray(bvp[:, None]),
    }
    in_maps = []
    for j in range(F):
        p, half = j // 2, j % 2
        fa, fb = p, F - 1 - p
        c0a = fa * HW + half * S
        c0b = fb * HW + half * S
        na, nb = 2 * (fa + 1), 2 * (fb + 1)
        assert na + nb == KSTEPS
        cols = []
        for hf in range(na):
            cols.append(X8[:, hf * S:(hf + 1) * S])
        for hf in range(nb):
            cols.append(X8[:, hf * S:(hf + 1) * S])
        m = dict(common)
        m["xq8"] = _swizzle(
            np.concatenate([X8[:, c0a:c0a + S], X8[:, c0b:c0b + S]], axis=1))
        m["xkv8"] = _swizzle(np.concatenate(cols, axis=1))
        m["xres"] = _swizzle(
            np.concatenate([X[:, c0a:c0a + S], X[:, c0b:c0b + S]], axis=1))
        m["qoff"] = np.asarray(
            [[0] * (na // 2) + [S] * (nb // 2)], np.int32
        )
        in_maps.append(m)
    return in_maps


def kernel(x, gamma, wq, bq, wk, bk, wv, bv, wp, bp, _trace=False):
    nc = _build()
    in_maps = _prep_inputs(x, gamma, wq, bq, wk, bk, wv, bv, wp, bp)
    kwargs = {}
    if _trace:
        kwargs = dict(trace=True, trace_cores=list(range(F)))
    r = run_bass_kernel_spmd(nc, in_maps, core_ids=list(range(F)), **kwargs)
    out = np.empty((1, C, F, HW), np.float32)
    for j in range(F):
        p, half = j // 2, j % 2
        fa, fb = p, F - 1 - p
        res = r.results[j]["out"]
        out[0, :, fa, half * S:half * S + S] = res[:, 0:S]
        out[0, :, fb, half * S:half * S + S] = res[:, S:Q]
    out = out.reshape(1, C, F, 32, 32)
    kernel._last_results = r
    return out


# revision 4
# speedup vs baseline: 1.2476x; 1.0138x over previous
"""Block-causal attention block (RMSnorm + QKV + frame-causal attention + proj)
on 8 TRN2 NeuronCores — fp8e4 DoubleRow, v11.

Sharding: sequence-parallel over the 8 frames — core j owns 512 queries of
frame p=j//2 (half j%2) and 512 of frame 7-p, and streams the 18 causal kv
half-blocks (512 tokens each) those two query halves attend to.  A per-pair
qoff input steers scores/O into the right query half (one uniform SPMD
program for all cores).

v3 structure (all heavy matmuls fp8e4 DoubleRow, K=256/matmul):
- x arrives pre-quantized to fp8 from the host; the residual re-loads fp32.
- kv tokens are never normalized explicitly: K is cast with a constant
  1/sqrt(C) scale, and the exact per-token 1/||x|| lands in the exp scale
  (per-partition AP) and the V psum->fp8 cast scale.  Only the 2 query slabs
  get a broadcast-multiply normalization.
- per-token rsqrt(sum x^2) is computed with the bit-trick + one Newton step
  on [128,4]-shaped tiles (transposed via 4 tiny SBUF DMAs) — no activation
  tables, so the scalar engine only ever loads the Exp table.
- O/den accumulate in PSUM across each query pair (start/stop spanning two
  kv steps, groups interleaved across banks) halving the SBUF accumulations.
- engine split: PE matmuls; gpsimd squares x (no PSUM port, SBUF-only ops);
  scalar does exp and the V casts; DVE does K/q casts, rsqrt, O/den drains.

Host-side folds: gamma*sqrt(C) and x32 into wq/wk/wv, x32 into wp, bv
through wp into the output bias, bk dropped (cancels in softmax).
Accuracy vs fp32 reference ~4e-4 (tolerance 2e-2).
"""

import sys

import numpy as np

sys.path.insert(0, "/opt/trn_rl_repo")

import ml_dtypes

import concourse.bacc as bacc
import concourse.bass as bass  # noqa: F401
import concourse.tile as tile
from concourse import mybir
from concourse.bass_utils import run_bass_kernel_spmd

C = 512
CC = C // 128          # 4 channel chunks of 128
KK = 2                 # 2 DoubleRow contraction chunks of 256
F = 8                  # frames
HW = 1024              # tokens per frame
SEQ = F * HW           # 8192
S = 512                # kv columns processed per step
KSTEPS = 18            # kv half-steps per core (perfectly balanced)
NPAIRS = KSTEPS // 2
SEQF = KSTEPS * S      # folded kv stream width
Q = 1024               # queries per core (two halves: one early, one late frame)
QH = Q // S            # 2 query halves
WS = 32.0              # fp8 range scale folded into wq/wk/wv/wp
SCALE = 1.0 / float(np.sqrt(C))
KD = 1.0 / float(np.sqrt(C))        # constant K-cast descale (nominal 1/||x||)
EXPSCALE = SCALE / (WS * WS)
RSQE_C = EXPSCALE * float(np.sqrt(C))  # exp scale = RSQE_C * rsq(token)
DESCALE = 1.0 / (WS * WS)
QUAKE_C = 0x5F3759DF
DYN_PAIRS = (1, 2, 3)
PREFETCH_AT = {0: (2,), 2: (3,)}

F32 = mybir.dt.float32
F8 = mybir.dt.float8e4
I32 = mybir.dt.int32
E4 = ml_dtypes.float8_e4m3
DR = mybir.MatmulPerfMode.DoubleRow
Act = mybir.ActivationFunctionType
Alu = mybir.AluOpType

_cached = {}


def _build():
    if "nc" in _cached:
        return _cached["nc"]

    nc = bacc.Bacc()
    xq_d = nc.dram_tensor("xq8", [128, QH * CC * S], F8, kind="ExternalInput")
    xkv_d = nc.dram_tensor("xkv8", [128, KSTEPS * CC * S], F8, kind="ExternalInput")
    xres_d = nc.dram_tensor("xres", [128, QH * CC * S], F32, kind="ExternalInput")
    qoff_d = nc.dram_tensor("qoff", [1, NPAIRS], I32, kind="ExternalInput")
    wq_d = nc.dram_tensor("wq8", [128, KK * 2 * C], F8, kind="ExternalInput")
    wk_d = nc.dram_tensor("wk8", [128, KK * 2 * C], F8, kind="ExternalInput")
    wv_d = nc.dram_tensor("wv8", [128, KK * 2 * C], F8, kind="ExternalInput")
    wp_d = nc.dram_tensor("wp8", [128, KK * 2 * C], F8, kind="ExternalInput")
    bq_d = nc.dram_tensor("bq", [C, 1], F32, kind="ExternalInput")
    bvp_d = nc.dram_tensor("bvp", [C, 1], F32, kind="ExternalInput")
    out_d = nc.dram_tensor("out", [C, Q], F32, kind="ExternalOutput")

    with tile.TileContext(nc) as tc:
        with (
            tc.tile_pool(name="const", bufs=1) as const,
            tc.tile_pool(name="persist", bufs=1) as persist,
            tc.tile_pool(name="xload", bufs=6) as xload,
            tc.tile_pool(name="norm", bufs=2) as norm,
            tc.tile_pool(name="kv", bufs=3) as kvpool,
            tc.tile_pool(name="ppool", bufs=3) as ppool,
            tc.tile_pool(name="psum_mm", bufs=3, space="PSUM") as psum_mm,
            tc.tile_pool(name="psum_o", bufs=4, space="PSUM") as psum_o,
            tc.tile_pool(name="psum_den", bufs=1, space="PSUM") as psum_den,
        ):
            NU_ALL = QH + KSTEPS
            # ---- prefetch ALL x slabs upfront (40KB/partition): the sync
            # queue then never delays a load behind the per-unit transpose
            # DMAs, so the gpsimd x^2 chain always has its input early ----
            xpre = {}
            for u0 in range(QH):
                xt0 = xload.tile([128, CC, S], F8, tag="xt", name="xt", bufs=NU_ALL)
                nc.sync.dma_start(
                    out=xt0[:], in_=xq_d[:, u0 * CC * S:(u0 + 1) * CC * S],
                )
                xpre[u0] = xt0
            for u0 in range(QH, NU_ALL):
                xt0 = xload.tile([128, CC, S], F8, tag="xt", name="xt", bufs=NU_ALL)
                nc.sync.dma_start(
                    out=xt0[:],
                    in_=xkv_d[:, (u0 - QH) * CC * S:(u0 - QH + 1) * CC * S],
                )
                xpre[u0] = xt0

            # ---- constants / weights ----
            wq_sb = const.tile([128, KK, 2, CC, 128], F8, tag="wq", name="wq_sb")
            wk_sb = const.tile([128, KK, 2, CC, 128], F8, tag="wk", name="wk_sb")
            wv_sb = const.tile([128, KK, 2, CC, 128], F8, tag="wv", name="wv_sb")
            wp_sb = const.tile([128, KK, 2, CC, 128], F8, tag="wp", name="wp_sb")
            for w_sb, w_d in (
                (wq_sb, wq_d), (wk_sb, wk_d), (wv_sb, wv_d), (wp_sb, wp_d),
            ):
                nc.sync.dma_start(out=w_sb[:], in_=w_d[:])
            bq_sb = const.tile([128, CC], F32, tag="bq", name="bq_sb")
            bvp_sb = const.tile([128, CC], F32, tag="bvp", name="bvp_sb")
            for b_sb, b_d in ((bq_sb, bq_d), (bvp_sb, bvp_d)):
                for ci in range(CC):
                    nc.sync.dma_start(
                        out=b_sb[:, ci:ci + 1],
                        in_=b_d[ci * 128:(ci + 1) * 128, :],
                    )
            qoff_sb = const.tile([1, NPAIRS], I32, tag="qoff", name="qoff_sb")
            nc.sync.dma_start(out=qoff_sb[:], in_=qoff_d[:])
            # DR "ones" stationary: [128, 2, 128] with ones in column m=0 only,
            # so ones-reductions land in psum partition 0
            ones_f = const.tile([128, 2, 128], F32, tag="ones_f", name="ones_f")
            nc.vector.memset(ones_f[:], 0.0)
            nc.vector.memset(ones_f[:, :, 0:1], 1.0)
            ones8 = const.tile([128, 2, 128], F8, tag="ones8", name="ones8")
            nc.vector.tensor_copy(ones8[:], ones_f[:])
            # [1,128] fp32 ones: PE-side partition broadcast via K=1 matmul
            ones1 = const.tile([1, 128], F32, tag="ones1", name="ones1")
            nc.vector.memset(ones1[:], 1.0)

            # PE warmup: back-to-back fp8 DR matmuls so the HAM clock gate
            # opens (4/8 -> 8/8) before the real matmul stream begins
            warm_f = norm.tile([128, 2, S], F32, tag="lnb", name="warm_f")
            nc.vector.memset(warm_f[:], 0.0)
            warm_r = norm.tile([128, 2, S], F8, tag="warm8", name="warm_r")
            nc.vector.tensor_copy(warm_r[:], warm_f[:])
            warm_ps = psum_mm.tile([128, S], F32, tag="mm", name="warm_ps")
            for wi in range(24):
                nc.tensor.matmul(
                    warm_ps[:], ones8[:], warm_r[:],
                    start=(wi == 0), stop=(wi == 23), perf_mode=DR,
                )

            # ---- persistent q-side tiles ----
            q8_sb = persist.tile([128, CC, Q], F8, tag="qT", name="q8_sb")
            o_sb = persist.tile([128, CC, Q], F32, tag="o", name="o_sb")
            nc.vector.memset(o_sb[:], 0.0)
            den_sb = persist.tile([1, Q], F32, tag="den_sb", name="den_sb")
            nc.vector.memset(den_sb[:], 0.0)

            def quake(out_f32, src_f32, shape, tagp):
                """rsqrt via bit trick + 1 Newton step; all DVE, no tables."""
                ti = norm.tile(shape, I32, tag=tagp + "i", name="q_ti")
                nc.vector.tensor_scalar(
                    out=ti[:], in0=src_f32.bitcast(I32), scalar1=1, scalar2=None,
                    op0=Alu.logical_shift_right,
                )
                nc.vector.tensor_scalar(
                    out=ti[:], in0=ti[:], scalar1=-1, scalar2=QUAKE_C,
                    op0=Alu.mult, op1=Alu.add,
                )
                y0 = ti[:].bitcast(F32)
                h = norm.tile(shape, F32, tag=tagp + "h", name="q_h")
                nc.vector.tensor_mul(h[:], src_f32, y0)
                nc.vector.tensor_mul(h[:], h[:], y0)
                nc.vector.tensor_scalar(
                    out=h[:], in0=h[:], scalar1=-0.5, scalar2=1.5,
                    op0=Alu.mult, op1=Alu.add,
                )
                nc.vector.tensor_mul(out_f32, y0, h[:])

            # ---- single-load pipeline, LAG=2: unit u loads its fp8 x slab,
            # squares it (gpsimd), sums channels on the PE, and derives the
            # per-token 1/||x|| factors; two units later the slab feeds the
            # projections / attention step ----
            UNITS = list(range(NU_ALL))
            LAG = 2
            xts = {}
            rsq1s = {}
            rsqs = {}
            pair_state = {}

            xsqs = {}

            def stats_load(u):
                xt = xpre.pop(u)
                xts[u] = xt
                xsq = norm.tile([128, CC, S], F8, tag="xsq", name="xsq")
                for ci in range(CC):
                    nc.gpsimd.tensor_mul(xsq[:, ci, :], xt[:, ci, :], xt[:, ci, :])
                xsqs[u] = xsq

            def stats_finish(u):
                xsq = xsqs.pop(u)
                ss_ps = psum_mm.tile([128, S], F32, tag="mm", name="ss_ps")
                for k in range(KK):
                    nc.tensor.matmul(
                        ss_ps[:], ones8[:], xsq[:, 2 * k:2 * k + 2, :],
                        start=(k == 0), stop=(k == KK - 1), perf_mode=DR,
                    )
                if u < QH:
                    rsq1 = norm.tile([1, S], F32, tag="rsq1", name="rsq1", bufs=3)
                    quake(rsq1[:], ss_ps[0:1, :], [1, S], "qq")
                    rsq1s[u] = rsq1
                else:
                    ss_sb = norm.tile([1, S], F32, tag="sscp", name="ss_sb", bufs=3)
                    nc.scalar.copy(ss_sb[:], ss_ps[0:1, :])
                    ssT = norm.tile([128, CC], F32, tag="ssT", name="ssT", bufs=3)
                    for kp in range(CC):
                        nc.sync.dma_start(
                            out=ssT[:, kp:kp + 1],
                            in_=ss_sb[0:1, kp * 128:(kp + 1) * 128],
                        )
                    rsqT = norm.tile([128, CC], F32, tag="rsqT", name="rsqT", bufs=3)
                    quake(rsqT[:], ssT[:], [128, CC], "qk")
                    rsqE = norm.tile([128, CC], F32, tag="rsqE", name="rsqE", bufs=3)
                    nc.vector.tensor_scalar_mul(rsqE[:], rsqT[:], RSQE_C)
                    rsqs[u] = (rsqT, rsqE)

            def work_part(u, hook):
                if u < QH:
                    qh = u
                    # normalize the query slab and project; the 1/||x|| row is
                    # partition-broadcast by a K=1 fp32 matmul (PE) so the
                    # gpsimd x^2 backlog never gates the ramp
                    rsqb = psum_o.tile([128, S], F32, tag="o", name="rsqb_ps")
                    nc.tensor.matmul(
                        rsqb[:], ones1[:], rsq1s.pop(u)[:], start=True, stop=True,
                    )
                    xn = norm.tile([128, CC, S], F8, tag="xn", name="xn")
                    xt = xts.pop(u)
                    for ci in range(CC):
                        nc.vector.tensor_mul(xn[:, ci, :], xt[:, ci, :], rsqb[:])
                    for co in range(CC):
                        q_ps = psum_mm.tile([128, S], F32, tag="mm", name="q_ps")
                        for k in range(KK):
                            nc.tensor.matmul(
                                q_ps[:],
                                wq_sb[:, k, :, co, :],
                                xn[:, 2 * k:2 * k + 2, :],
                                start=(k == 0), stop=(k == KK - 1), perf_mode=DR,
                            )
                        nc.vector.tensor_scalar_add(
                            q8_sb[:, co, qh * S:(qh + 1) * S], q_ps[:],
                            bq_sb[:, co:co + 1],
                        )
                    hook()
                    if qh == QH - 1:
                        for rr in (1,):
                            roff = nc.values_load(
                                qoff_sb[0:1, rr:rr + 1],
                                engines=[mybir.EngineType.DVE],
                                min_val=0, max_val=S,
                                skip_runtime_bounds_check=True,
                            )
                            q8r = kvpool.tile(
                                [128, CC, S], F8, tag="qcur", name="q8cur",
                                bufs=len(DYN_PAIRS),
                            )
                            for ci in range(CC):
                                nc.vector.tensor_copy(
                                    q8r[:, ci, :],
                                    q8_sb[:, ci, bass.ds(roff, S)],
                                )
                            pair_state[rr] = (roff, q8r)
                    return

                t = u - QH
                r, phase = divmod(t, 2)
                xt = xts.pop(u)
                rsqT, rsqE = rsqs.pop(u)
                # Query half per pair: with na in {2,4,6,8}, pair 0 always
                # targets the early half and pairs 4+ the late half; only
                # pairs 1-3 vary per core.  Static pairs read q8_sb directly
                # (legal strided AP); dynamic pairs use the q8cur staged by
                # the previous pair (prefetched before the PSUM drains).
                dynamic = r in DYN_PAIRS
                if phase == 0:
                    for rr in PREFETCH_AT.get(t, ()):
                        roff = nc.values_load(
                            qoff_sb[0:1, rr:rr + 1],
                            engines=[mybir.EngineType.DVE],
                            min_val=0, max_val=S,
                            skip_runtime_bounds_check=True,
                        )
                        q8r = kvpool.tile(
                            [128, CC, S], F8, tag="qcur", name="q8cur",
                            bufs=len(DYN_PAIRS),
                        )
                        for ci in range(CC):
                            nc.vector.tensor_copy(
                                q8r[:, ci, :],
                                q8_sb[:, ci, bass.ds(roff, S)],
                            )
                        pair_state[rr] = (roff, q8r)
                    if dynamic:
                        off, q8cur = pair_state.pop(r)
                        qsrc, qbase = q8cur, 0
                    else:
                        off, qsrc = None, q8_sb
                        qbase = 0 if r == 0 else S
                    dn_ps = psum_den.tile([128, S], F32, tag="den", name="dn_ps")
                    o_pss = [
                        psum_o.tile([128, S], F32, tag="o", name="o_ps")
                        for _ in range(CC)
                    ]
                    pair_state.update(off=off, qsrc=qsrc, qbase=qbase,
                                      dn=dn_ps, o=o_pss)
                else:
                    off = pair_state["off"]
                    qsrc = pair_state["qsrc"]
                    qbase = pair_state["qbase"]
                    dn_ps = pair_state["dn"]
                    o_pss = pair_state["o"]

                # k^T projection on raw fp8 x; constant 1/sqrt(C) range cast
                kT = kvpool.tile([128, CC, S], F8, tag="kT", name="kT")
                for co in range(CC):
                    k_ps = psum_mm.tile([128, S], F32, tag="mm", name="k_ps")
                    for k in range(KK):
                        nc.tensor.matmul(
                            k_ps[:],
                            wk_sb[:, k, :, co, :],
                            xt[:, 2 * k:2 * k + 2, :],
                            start=(k == 0), stop=(k == KK - 1), perf_mode=DR,
                        )
                    nc.scalar.activation(
                        kT[:, co, :], k_ps[:], Act.Copy, scale=KD,
                    )

                # v projection on raw fp8 x; exact per-token 1/||x|| cast scale
                v_sb = kvpool.tile([128, S // 128, C], F8, tag="v", name="v_sb")
                for kp in range(S // 128):
                    v_ps = psum_mm.tile([128, C], F32, tag="mm", name="v_ps")
                    for k in range(KK):
                        nc.tensor.matmul(
                            v_ps[:],
                            xt[:, 2 * k:2 * k + 2, kp * 128:(kp + 1) * 128],
                            wv_sb[:, k, :, :, :],
                            start=(k == 0), stop=(k == KK - 1), perf_mode=DR,
                        )
                    nc.scalar.activation(
                        v_sb[:, kp, :], v_ps[:], Act.Copy,
                        scale=rsqT[:, kp:kp + 1],
                    )

                # scores S^T = K Q^T; P = exp(S^T * scale(token)) in fp8
                p_sb = ppool.tile([128, S // 128, S], F8, tag="p", name="p_sb")
                for kp in range(S // 128):
                    s_ps = psum_mm.tile([128, S], F32, tag="mm", name="s_ps")
                    for k in range(KK):
                        nc.tensor.matmul(
                            s_ps[:],
                            kT[:, 2 * k:2 * k + 2, kp * 128:(kp + 1) * 128],
                            qsrc[:, 2 * k:2 * k + 2, qbase:qbase + S],
                            start=(k == 0), stop=(k == KK - 1), perf_mode=DR,
                        )
                    nc.scalar.activation(
                        p_sb[:, kp, :], s_ps[:], Act.Exp, bias=0.0,
                        scale=rsqE[:, kp:kp + 1],
                    )

                hook()

                # den and O accumulate in PSUM across the pair; groups
                # interleave across banks (hence skip_group_check)
                for k in range(KK):
                    nc.tensor.matmul(
                        dn_ps[:], ones8[:], p_sb[:, 2 * k:2 * k + 2, :],
                        start=(phase == 0 and k == 0),
                        stop=(phase == 1 and k == KK - 1),
                        perf_mode=DR, skip_group_check=True,
                    )
                    for co in range(CC):
                        nc.tensor.matmul(
                            o_pss[co][:],
                            v_sb[:, 2 * k:2 * k + 2, co * 128:(co + 1) * 128],
                            p_sb[:, 2 * k:2 * k + 2, :],
                            start=(phase == 0 and k == 0),
                            stop=(phase == 1 and k == KK - 1),
                            perf_mode=DR, skip_group_check=True,
                        )
                if phase == 1:
                    if off is None:
                        nc.vector.tensor_add(
                            den_sb[:, qbase:qbase + S],
                            den_sb[:, qbase:qbase + S],
                            dn_ps[0:1, :],
                        )
                        for co in range(CC):
                            nc.vector.tensor_add(
                                o_sb[:, co, qbase:qbase + S],
                                o_sb[:, co, qbase:qbase + S],
                                o_pss[co][:],
                            )
                    else:
                        nc.vector.tensor_add(
                            den_sb[:, bass.ds(off, S)],
                            den_sb[:, bass.ds(off, S)],
                            dn_ps[0:1, :],
                        )
                        for co in range(CC):
                            nc.vector.tensor_add(
                                o_sb[:, co, bass.ds(off, S)],
                                o_sb[:, co, bass.ds(off, S)],
                                o_pss[co][:],
                            )

            NU = len(UNITS)
            for i in range(NU + LAG):
                if i < NU:
                    stats_load(i)
                if i < LAG:
                    stats_finish(i)
                    if i == LAG - 1:
                        warm_ps2 = psum_mm.tile([128, S], F32, tag="mm", name="warm_ps2")
                        for wi in range(48):
                            nc.tensor.matmul(
                                warm_ps2[:], ones8[:], warm_r[:],
                                start=(wi == 0), stop=(wi == 47), perf_mode=DR,
                            )
                else:
                    hook = (lambda i=i: stats_finish(i)) if i < NU else (lambda: None)
                    work_part(i - LAG, hook)

            # ---- finalize: normalize, project, residual ----
            rdbs = []
            for qh in range(QH):
                rd = norm.tile([1, S], F32, tag="rn", name="rd", bufs=2)
                nc.vector.reciprocal_approx_fast(
                    out=rd[:], in_=den_sb[:, qh * S:(qh + 1) * S],
                )
                rdb_ps = psum_o.tile([128, S], F32, tag="o", name="rdb_ps")
                nc.tensor.matmul(
                    rdb_ps[:], ones1[:], rd[:], start=True, stop=True,
                )
                rdbs.append(rdb_ps)
            for qh in range(QH):
                # o_n := o * (1/den) for this half, cast to fp8
                on_sb = ppool.tile([128, CC, S], F8, tag="on", name="on_sb", bufs=2)
                for ci in range(CC):
                    nc.vector.tensor_mul(
                        on_sb[:, ci, :], o_sb[:, ci, qh * S:(qh + 1) * S],
                        rdbs[qh][:],
                    )
                xr = xload.tile([128, CC, S], F32, tag="xr", name="xr", bufs=2)
                nc.sync.dma_start(
                    out=xr[:],
                    in_=xres_d[:, qh * CC * S:(qh + 1) * CC * S],
                )
                for co in range(CC):
                    pr_ps = psum_mm.tile([128, S], F32, tag="mm", name="pr_ps")
                    for k in range(KK):
                        nc.tensor.matmul(
                            pr_ps[:],
                            wp_sb[:, k, :, co, :],
                            on_sb[:, 2 * k:2 * k + 2, :],
                            start=(k == 0), stop=(k == KK - 1), perf_mode=DR,
                        )
                    prs = norm.tile([128, S], F32, tag="prs", name="prs")
                    nc.scalar.mul(prs[:], pr_ps[:], DESCALE)
                    res = norm.tile([128, S], F32, tag="res", name="res")
                    nc.vector.scalar_tensor_tensor(
                        out=res[:],
                        in0=prs[:],
                        scalar=bvp_sb[:, co:co + 1],
                        in1=xr[:, co, :],
                        op0=Alu.add,
                        op1=Alu.add,
                    )
                    nc.sync.dma_start(
                        out=out_d[co * 128:(co + 1) * 128, qh * S:(qh + 1) * S],
                        in_=res[:],
                    )

    nc.finalize()
    _cached["nc"] = nc
    return nc


def _dr_layout(wt):
    """[C_in, C_out] f32 -> [128, KK*2*C_out] fp8 in DoubleRow stationary
    order: [p, k, i, co, m] = wt[k*256 + i*128 + p, co*128 + m]."""
    t = wt.reshape(KK, 2, 128, CC, 128).transpose(2, 0, 1, 3, 4)
    return np.ascontiguousarray(t.reshape(128, KK * 2 * C)).astype(E4)


def _swizzle(xcs):
    """[C, n*S] -> [128, n*CC*S]: slab n contiguous as [CC, S] per partition."""
    n = xcs.shape[1] // S
    t = xcs.reshape(CC, 128, n, S).transpose(1, 2, 0, 3)
    return np.ascontiguousarray(t.reshape(128, n * CC * S))


def _prep_inputs(x, gamma, wq, bq, wk, bk, wv, bv, wp, bp):
    x = np.asarray(x, np.float32)
    X = np.ascontiguousarray(x[0].reshape(C, SEQ))
    X8 = X.astype(E4)
    g = (np.asarray(gamma, np.float32) * np.float32(np.sqrt(C))).astype(np.float32)
    wq = np.asarray(wq, np.float32)
    wk = np.asarray(wk, np.float32)
    wv = np.asarray(wv, np.float32)
    wp = np.asarray(wp, np.float32)
    bq = np.asarray(bq, np.float32)
    bv = np.asarray(bv, np.float32)
    bp = np.asarray(bp, np.float32)
    wq8 = _dr_layout((wq * g[None, :]).T * WS)
    wk8 = _dr_layout((wk * g[None, :]).T * WS)
    wv8 = _dr_layout((wv * g[None, :]).T * WS)
    wp8 = _dr_layout(wp.T * WS)
    bvp = (bp + wp @ bv).astype(np.float32)

    common = {
        "wq8": wq8, "wk8": wk8, "wv8": wv8, "wp8": wp8,
        "bq": np.ascontiguousarray((bq * WS)[:, None]).astype(np.float32),
        "bvp": np.ascontiguousarray(bvp[:, None]),
    }
    in_maps = []
    for j in range(F):
        p, half = j // 2, j % 2
        fa, fb = p, F - 1 - p
        c0a = fa * HW + half * S
        c0b = fb * HW + half * S
        na, nb = 2 * (fa + 1), 2 * (fb + 1)
        assert na + nb == KSTEPS
        cols = []
        for hf in range(na):
            cols.append(X8[:, hf * S:(hf + 1) * S])
        for hf in range(nb):
            cols.append(X8[:, hf * S:(hf + 1) * S])
        m = dict(common)
        m["xq8"] = _swizzle(
            np.concatenate([X8[:, c0a:c0a + S], X8[:, c0b:c0b + S]], axis=1))
        m["xkv8"] = _swizzle(np.concatenate(cols, axis=1))
        m["xres"] = _swizzle(
            np.concatenate([X[:, c0a:c0a + S], X[:, c0b:c0b + S]], axis=1))
        m["qoff"] = np.asarray(
            [[0] * (na // 2) + [S] * (nb // 2)], np.int32
        )
        in_maps.append(m)
    return in_maps


def kernel(x, gamma, wq, bq, wk, bk, wv, bv, wp, bp, _trace=False):
    nc = _build()
    in_maps = _prep_inputs(x, gamma, wq, bq, wk, bk, wv, bv, wp, bp)
    kwargs = {}
    if _trace:
        kwargs = dict(trace=True, trace_cores=list(range(F)))
    r = run_bass_kernel_spmd(nc, in_maps, core_ids=list(range(F)), **kwargs)
    out = np.empty((1, C, F, HW), np.float32)
    for j in range(F):
        p, half = j // 2, j % 2
        fa, fb = p, F - 1 - p
        res = r.results[j]["out"]
        out[0, :, fa, half * S:half * S + S] = res[:, 0:S]
        out[0, :, fb, half * S:half * S + S] = res[:, S:Q]
    out = out.reshape(1, C, F, 32, 32)
    kernel._last_results = r
    return out
